# revision 1
# baseline (speedup 1.0000x reference)
"""Mixer (token-mix + channel-mix MLP) kernel for 8 TRN2 NeuronCores.

Strategy (expert-style parallel over the group axes):
  Phase 1 (C-sharded): core m owns channels Cm=[32m,32m+32). LN1 is folded into
  the per-channel PE transpose (augmented matmul whose moving operand is
  [diag(rstd); -mu*rstd] next to an identity, yielding both (x-mu)*rstd and x
  transposed in one matmul); g1/be1 are folded into the fc1 weights/bias on
  the host. Token-mix fc1+fc2 run in bf16 with per-channel [128,128]
  stationary blocks in transposed layout ([feature, batch]); accumulation is
  fp32 in PSUM. The residual u = x + tok is accumulated in fp32 into an SBUF
  staging buffer; LN2 stats come from an fp32 gram matmul on uT with a ones
  column appended ([sum u*u | sum u]).
  AllToAll: the u staging buffer + LN2 stats ship so core k owns patches
  Nk=[32k,32k+32) with all 256 channels.
  Phase 2 (N-sharded): yn = (u-mu2)*rstd2 is materialized (bf16) on the
  receive side from the shipped stats (g2/be2 folded into cw1/bias on host),
  then the channel-mix fc1+fc2 and final fp32 residual run per patch; output
  stays in [patch, channel, batch] layout and the host transposes it back.
"""
import sys
import numpy as np

sys.path.insert(0, "/opt/trn_rl_repo")

import ml_dtypes
import concourse.bass as bass
import concourse.bacc as bacc
import concourse.tile as tile
from concourse import mybir
from concourse.bass_utils import run_bass_kernel_spmd

F32 = mybir.dt.float32
BF16 = mybir.dt.bfloat16
NCORE = 8
B, C, N = 64, 256, 256
CL = C // NCORE   # 32 local channels (phase 1)
NL = N // NCORE   # 32 local patches (phase 2)
EPS = 1e-5
GELU = mybir.ActivationFunctionType.Gelu

HC = CL // 2                     # 16 channels per collective half
A_ELEMS = HC * NL * B            # 32768 u elems per half-block
ST_ELEMS = B * CL * 2            # 4096 stats elems per block
B_ELEMS = A_ELEMS + ST_ELEMS     # second half carries the stats


def build_program(gelu_func=GELU, mmdt=BF16, skip_b2=False, skip_bc2=False):
    nc = bacc.Bacc("TRN2", target_bir_lowering=False, debug=False,
                   enable_asserts=True, num_devices=NCORE)

    x_in = nc.dram_tensor("x_sh", [B, CL, N], F32, kind="ExternalInput")
    wt_in = nc.dram_tensor("wt", [CL, 128, 4, N], mmdt, kind="ExternalInput")
    ct_in = nc.dram_tensor("ct", [NL, 128, 4, C], mmdt, kind="ExternalInput")
    b1t_in = nc.dram_tensor("b1t", [128, 2, CL], F32, kind="ExternalInput")
    b2t_in = nc.dram_tensor("b2t", [128, 2, CL], F32, kind="ExternalInput")
    bc1t_in = nc.dram_tensor("bc1t", [128, 2, NL], F32, kind="ExternalInput")
    bc2t_in = nc.dram_tensor("bc2t", [128, 2, NL], F32, kind="ExternalInput")
    id64_in = nc.dram_tensor("id64", [64, 64], F32, kind="ExternalInput")
    idx_in = nc.dram_tensor("idx65", [65, 64], F32, kind="ExternalInput")

    ybuf = nc.dram_tensor("ybuf", [NL, C, B], F32, kind="ExternalOutput")

    with tile.TileContext(nc) as tc:
        with tc.tile_pool(name="const", bufs=1) as const, \
             tc.tile_pool(name="wpool", bufs=3) as wpool, \
             tc.tile_pool(name="act", bufs=4) as act, \
             tc.tile_pool(name="small", bufs=4) as small, \
             tc.tile_pool(name="dram", bufs=1, space="DRAM") as dram, \
             tc.tile_pool(name="ps", bufs=2, space="PSUM") as ps:

            # two collective halves (by channel) so the first all-to-all
            # overlaps the tail of phase-1 compute; block layout is c-major
            # [c_loc, nl, b]
            send_a = dram.tile([NCORE, A_ELEMS], F32)
            recv_a = dram.tile([NCORE, A_ELEMS], F32)
            send_b = dram.tile([NCORE, B_ELEMS], F32)
            recv_b = dram.tile([NCORE, B_ELEMS], F32)
            halves = [(send_a, recv_a, A_ELEMS), (send_b, recv_b, B_ELEMS)]

            def send_u_view(h, j):
                # [32nl, 16c, 64b]-ordered AP into half h's dest-j block
                st, _, blk = halves[h]
                return bass.AP(tensor=st.tensor, offset=j * blk,
                               ap=[[B, NL], [NL * B, HC], [1, B]])

            def send_st_view(j):
                # [64b, 32c, 2] stats region of send_b's dest-j block
                return bass.AP(tensor=send_b.tensor,
                               offset=j * B_ELEMS + A_ELEMS,
                               ap=[[CL * 2, B], [2, CL], [1, 2]])

            def recv_u_view(h, j):
                # [16c, 32nl, 64b] view of src-core j's u in half h
                _, rt, blk = halves[h]
                return bass.AP(tensor=rt.tensor, offset=j * blk,
                               ap=[[NL * B, HC], [B, NL], [1, B]])

            def recv_st_view(j, comp):
                # [32c, 64b] view of src-core j's stats (layout [b, c, t])
                return bass.AP(tensor=recv_b.tensor,
                               offset=j * B_ELEMS + A_ELEMS + comp,
                               ap=[[2, CL], [CL * 2, B]])

            # ---- constants / persistent tiles ----
            x_aug = const.tile([65, CL, N], F32)      # rows 0-63 = x[b], row 64 = 1
            nc.sync.dma_start(out=x_aug[0:64, :, :], in_=x_in[:])
            nc.vector.memset(x_aug[64:65, :, :], 1.0)
            id64 = const.tile([64, 64], F32)
            nc.sync.dma_start(out=id64[:], in_=id64_in[:])
            idx65 = const.tile([65, 64], F32)
            nc.sync.dma_start(out=idx65[:], in_=idx_in[:])
            b1t = const.tile([128, 2, CL], F32)
            nc.sync.dma_start(out=b1t[:], in_=b1t_in[:])
            b2t = const.tile([128, 2, CL], F32)
            nc.sync.dma_start(out=b2t[:], in_=b2t_in[:])
            bc1t = const.tile([128, 2, NL], F32)
            nc.sync.dma_start(out=bc1t[:], in_=bc1t_in[:])
            bc2t = const.tile([128, 2, NL], F32)
            nc.sync.dma_start(out=bc2t[:], in_=bc2t_in[:])
            eps64 = const.tile([64, 1], F32)
            nc.vector.memset(eps64[:], EPS)

            mv_all = const.tile([64, CL, 2], F32)     # LN1 mean/var
            rstd1_all = const.tile([64, CL], F32)
            nmr1_all = const.tile([64, CL], F32)      # -mu1*rstd1
            mu2_all = const.tile([64, CL], F32)
            var2_all = const.tile([64, CL], F32)
            rstd2_all = const.tile([64, CL], F32)
            nmr2_all = const.tile([64, CL], F32)      # -mu2*rstd2
            # per-channel moving operand: [diag(rstd1); -mu*rstd] | [I64; 0]
            movings2 = const.tile([65, CL, 2, 64], F32)
            u_stage = const.tile([128, 2, CL, 64], F32)   # uT for all channels
            ua = const.tile([128, 2, 65], F32)            # [u | 1] gram rhs
            nc.vector.memset(ua[:, :, 64:65], 1.0)
            sum_all = const.tile([64, CL], F32)           # sum_n u
            esq_all = const.tile([64, CL], F32)           # E[u^2]
            dg = const.tile([64, 64], F32)                # gram*mask scratch

            # ---- phase 1a: LN1 stats, per-channel so matmuls start early ----
            for c in range(CL):
                st6 = small.tile([64, 6], F32)
                nc.vector.bn_stats(out=st6[:], in_=x_aug[0:64, c, :])
                nc.vector.bn_aggr(out=mv_all[:, c, :], in_=st6[:])
                nc.scalar.activation(out=rstd1_all[:, c:c + 1],
                                     in_=mv_all[:, c, 1:2],
                                     func=mybir.ActivationFunctionType.Sqrt,
                                     bias=eps64[:], scale=1.0)
                nc.vector.reciprocal(out=rstd1_all[:, c:c + 1],
                                     in_=rstd1_all[:, c:c + 1])
            nc.vector.tensor_mul(out=nmr1_all[:], in0=mv_all[:, :, 0],
                                 in1=rstd1_all[:])
            nc.vector.tensor_scalar_mul(out=nmr1_all[:], in0=nmr1_all[:],
                                        scalar1=-1.0)
            for c in range(CL):
                nc.gpsimd.tensor_scalar_mul(out=movings2[0:64, c, 0, :],
                                            in0=id64[:],
                                            scalar1=rstd1_all[:, c:c + 1])
                nc.gpsimd.tensor_copy(out=movings2[:, c, 1, :], in_=idx65[:])
                # [64,1] column -> [1,64] row via partition-collapse DMA
                nc.gpsimd.dma_start(out=movings2[64:65, c, 0, :],
                                    in_=nmr1_all[:, c:c + 1])

            # ---- phase 1b: token mixing per channel ----
            for c in range(CL):
                w12 = wpool.tile([128, 4, N], mmdt, tag="w")
                nc.scalar.dma_start(out=w12[:], in_=wt_in[c])

                # zx[:, blk, 0:64] = (x-mu)*rstd transposed; [64:128] = x.T
                zx = ps.tile([128, 2, 128], F32, tag="zx")
                for blk in range(2):
                    nc.tensor.matmul(
                        zx[:, blk, :],
                        x_aug[:, c, blk * 128:(blk + 1) * 128],
                        movings2[:, c, :, :].rearrange("p a b -> p (a b)"),
                        start=True, stop=True)
                z_sb = act.tile([128, 2, 64], mmdt, tag="z")
                nc.vector.tensor_copy(out=z_sb[:], in_=zx[:, :, 0:64])

                hpre = ps.tile([128, 2, 64], F32, tag="hpre")
                for mb in range(2):
                    for nb in range(2):
                        nc.tensor.matmul(
                            hpre[:, mb, :],
                            w12[:, nb, mb * 128:(mb + 1) * 128],
                            z_sb[:, nb, :],
                            start=(nb == 0), stop=(nb == 1))
                hs = act.tile([128, 2, 64], mmdt, tag="h")
                for mb in range(2):
                    nc.scalar.activation(out=hs[:, mb, :], in_=hpre[:, mb, :],
                                         func=gelu_func, bias=b1t[:, mb, c:c + 1])

                tokp = ps.tile([128, 2, 64], F32, tag="tokp")
                for kb in range(2):
                    for mb in range(2):
                        nc.tensor.matmul(
                            tokp[:, kb, :],
                            w12[:, 2 + mb, kb * 128:(kb + 1) * 128],
                            hs[:, mb, :],
                            start=(mb == 0), stop=(mb == 1))
                # u = x.T + tok (+ tb2): DVE may read only one PSUM input/op
                if skip_b2:
                    nc.vector.tensor_copy(out=u_stage[:, :, c, :],
                                          in_=zx[:, :, 64:128])
                else:
                    for kb in range(2):
                        nc.vector.tensor_scalar(
                            out=u_stage[:, kb, c, :], in0=zx[:, kb, 64:128],
                            scalar1=b2t[:, kb, c:c + 1], scalar2=None,
                            op0=mybir.AluOpType.add)
                nc.vector.tensor_add(out=u_stage[:, :, c, :],
                                     in0=u_stage[:, :, c, :], in1=tokp[:])

                # LN2 stats: gram of uT with a ones column ([sum uu | sum u])
                nc.vector.tensor_copy(out=ua[:, :, 0:64], in_=u_stage[:, :, c, :])
                gs = ps.tile([64, 65], F32, tag="gs")
                for blk in range(2):
                    nc.tensor.matmul(gs[:], u_stage[:, blk, c, :], ua[:, blk, :],
                                     start=(blk == 0), stop=(blk == 1))
                nc.vector.tensor_mul(out=dg[:], in0=gs[:, 0:64], in1=id64[:])
                nc.vector.reduce_sum(out=esq_all[:, c:c + 1], in_=dg[:],
                                     axis=mybir.AxisListType.X)
                nc.vector.tensor_copy(out=sum_all[:, c:c + 1], in_=gs[:, 64:65])

            # ---- phase 1c: LN2 rstd batch + ship u and stats ----
            nc.vector.tensor_scalar(
                out=mu2_all[:], in0=sum_all[:], scalar1=1.0 / N, scalar2=None,
                op0=mybir.AluOpType.mult)
            nc.vector.tensor_scalar(
                out=esq_all[:], in0=esq_all[:], scalar1=1.0 / N, scalar2=None,
                op0=mybir.AluOpType.mult)
            nc.vector.tensor_mul(out=var2_all[:], in0=mu2_all[:],
                                 in1=mu2_all[:])
            nc.vector.tensor_sub(out=var2_all[:], in0=esq_all[:],
                                 in1=var2_all[:])
            nc.scalar.activation(out=rstd2_all[:], in_=var2_all[:],
                                 func=mybir.ActivationFunctionType.Sqrt,
                                 bias=eps64[:], scale=1.0)
            nc.vector.reciprocal(out=rstd2_all[:], in_=rstd2_all[:])
            nc.vector.tensor_mul(out=nmr2_all[:], in0=mu2_all[:],
                                 in1=rstd2_all[:])
            nc.vector.tensor_scalar_mul(out=nmr2_all[:], in0=nmr2_all[:],
                                        scalar1=-1.0)
            stats_il = const.tile([64, CL, 2], F32)
            nc.vector.tensor_copy(out=stats_il[:, :, 0], in_=nmr2_all[:])
            nc.vector.tensor_copy(out=stats_il[:, :, 1], in_=rstd2_all[:])
            for h in range(2):
                for blk in range(2):
                    for jr in range(4):
                        j = blk * 4 + jr
                        nc.sync.dma_start(
                            out=send_u_view(h, j),
                            in_=u_stage[jr * 32:(jr + 1) * 32, blk,
                                        h * HC:(h + 1) * HC, :])
            for j in range(NCORE):
                nc.sync.dma_start(out=send_st_view(j), in_=stats_il[:])

            # ---- collectives (half A can start while phase 1 finishes) ----
            nc.gpsimd.collective_compute(
                "AllToAll", mybir.AluOpType.bypass,
                replica_groups=[list(range(NCORE))],
                ins=[send_a.opt()], outs=[recv_a.opt()])
            nc.gpsimd.collective_compute(
                "AllToAll", mybir.AluOpType.bypass,
                replica_groups=[list(range(NCORE))],
                ins=[send_b.opt()], outs=[recv_b.opt()])

            # ---- phase 2a: normalization maps + u staging [c, b] ----
            nm_map = const.tile([128, 2, 64], F32)
            rs_map = const.tile([128, 2, 64], F32)
            recv_stage = const.tile([128, 2, NL, 64], F32)
            for ch in range(2):
                for jr in range(4):
                    j = 4 * ch + jr
                    nc.sync.dma_start(
                        out=nm_map[jr * 32:(jr + 1) * 32, ch, :],
                        in_=recv_st_view(j, 0))
                    nc.sync.dma_start(
                        out=rs_map[jr * 32:(jr + 1) * 32, ch, :],
                        in_=recv_st_view(j, 1))
                    for h in range(2):
                        nc.sync.dma_start(
                            out=recv_stage[jr * 32 + h * HC:
                                           jr * 32 + (h + 1) * HC, ch, :, :],
                            in_=recv_u_view(h, j))
            y_stage = const.tile([128, 2, NL, 64], F32)

            # ---- phase 2b: channel mixing per patch ----
            for nl in range(NL):
                c12 = wpool.tile([128, 4, C], mmdt, tag="w")
                nc.scalar.dma_start(out=c12[:], in_=ct_in[nl])

                u_n = recv_stage[:, :, nl, :]
                t2 = act.tile([128, 2, 64], F32, tag="t2")
                nc.vector.tensor_mul(out=t2[:], in0=u_n, in1=rs_map[:])
                yn = act.tile([128, 2, 64], mmdt, tag="yn")
                nc.vector.tensor_add(out=yn[:], in0=t2[:], in1=nm_map[:])

                h2p = ps.tile([128, 2, 64], F32,
                              tag=("hpre", "zx")[nl % 2])
                for ob in range(2):
                    for cb in range(2):
                        nc.tensor.matmul(
                            h2p[:, ob, :],
                            c12[:, cb, ob * 128:(ob + 1) * 128],
                            yn[:, cb, :],
                            start=(cb == 0), stop=(cb == 1))
                h2s = act.tile([128, 2, 64], mmdt, tag="h")
                for ob in range(2):
                    nc.scalar.activation(out=h2s[:, ob, :], in_=h2p[:, ob, :],
                                         func=gelu_func,
                                         bias=bc1t[:, ob, nl:nl + 1])

                chp = ps.tile([128, 2, 64], F32,
                              tag=("tokp", "gs")[nl % 2])
                for kb in range(2):
                    for ob in range(2):
                        nc.tensor.matmul(
                            chp[:, kb, :],
                            c12[:, 2 + ob, kb * 128:(kb + 1) * 128],
                            h2s[:, ob, :],
                            start=(ob == 0), stop=(ob == 1))
                if skip_bc2:
                    nc.vector.tensor_add(out=y_stage[:, :, nl, :], in0=chp[:],
                                         in1=u_n)
                else:
                    t3 = act.tile([128, 2, 64], F32, tag="t3")
                    for kb in range(2):
                        nc.vector.tensor_scalar(
                            out=t3[:, kb, :], in0=chp[:, kb, :],
                            scalar1=bc2t[:, kb, nl:nl + 1], scalar2=None,
                            op0=mybir.AluOpType.add)
                    nc.vector.tensor_add(out=y_stage[:, :, nl, :], in0=t3[:],
                                         in1=u_n)

            # ---- output: ybuf[nl, c, b] from y_stage[(k_lo), kb, nl, b] ----
            for kb in range(2):
                out_ap = bass.AP(tensor=ybuf,
                                 offset=kb * 128 * B,
                                 ap=[[B, 128], [C * B, NL], [1, B]])
                nc.sync.dma_start(out=out_ap, in_=y_stage[:, kb, :, :])

    nc.finalize()
    return nc


def prep_inputs(x, g1, be1, g2, be2, tw1, tb1, tw2, tb2, cw1, cb1, cw2, cb2,
                mmdt_np=ml_dtypes.bfloat16):
    """Host-side sharding + weight folding. Returns in_maps for the 8 cores."""
    f = np.float32
    bf = mmdt_np
    x = np.asarray(x, f)
    g1, be1, g2, be2 = (np.asarray(a, f) for a in (g1, be1, g2, be2))
    tw1, tb1, tw2, tb2 = (np.asarray(a, f) for a in (tw1, tb1, tw2, tb2))
    cw1, cb1, cw2, cb2 = (np.asarray(a, f) for a in (cw1, cb1, cw2, cb2))

    # token-mix fc1: fold g1 into weights, be1 into bias; lhsT layout [c, n, m]
    w1t = (tw1 * g1[None, None, :]).transpose(0, 2, 1)            # [C, N, N]
    bias1 = tb1 + np.einsum('n,cmn->cm', be1, tw1)                # [C, M]
    w2t = tw2.transpose(0, 2, 1)                                  # [c, m, k]
    t1r = w1t.reshape(C, 2, 128, N)
    t2r = w2t.reshape(C, 2, 128, N)
    wt = np.ascontiguousarray(
        np.stack([t1r[:, 0], t1r[:, 1], t2r[:, 0], t2r[:, 1]],
                 axis=2)).astype(bf)                              # [C, 128, 4, N]

    # channel-mix fc1: fold g2 (per-patch scalar) into cw1, be2 into bias
    c1t = (cw1 * g2[:, None, None]).transpose(0, 2, 1)            # [N, C, C]
    biasc1 = cb1 + be2[:, None] * cw1.sum(axis=2)                 # [N, O]
    c2t = cw2.transpose(0, 2, 1)                                  # [n, o, k]
    c1r = c1t.reshape(N, 2, 128, C)
    c2r = c2t.reshape(N, 2, 128, C)
    ct = np.ascontiguousarray(
        np.stack([c1r[:, 0], c1r[:, 1], c2r[:, 0], c2r[:, 1]],
                 axis=2)).astype(bf)                              # [N, 128, 4, C]

    id64 = np.eye(64, dtype=f)
    idx65 = np.vstack([np.eye(64, dtype=f), np.zeros((1, 64), f)])

    def fold_bias(bm):   # [G, 256] -> [128, 2, G]
        return np.ascontiguousarray(bm.T.reshape(2, 128, -1).transpose(1, 0, 2))

    in_maps = []
    for m in range(NCORE):
        cs = slice(m * CL, (m + 1) * CL)
        ns = slice(m * NL, (m + 1) * NL)
        in_maps.append({
            "x_sh": np.ascontiguousarray(x[:, cs, :]),
            "wt": np.ascontiguousarray(wt[cs]),
            "ct": np.ascontiguousarray(ct[ns]),
            "b1t": fold_bias(bias1[cs]),
            "b2t": fold_bias(tb2[cs]),
            "bc1t": fold_bias(biasc1[ns]),
            "bc2t": fold_bias(cb2[ns]),
            "id64": id64,
            "idx65": idx65,
        })
    return in_maps


def assemble_output(results):
    """results: list of per-core dicts with 'ybuf' [NL, C, B] -> y [B, C, N]."""
    y = np.empty((B, C, N), np.float32)
    for k in range(NCORE):
        y[:, :, k * NL:(k + 1) * NL] = results[k]["ybuf"].transpose(2, 1, 0)
    return y


_PROGRAMS = {}


def get_program(skip_b2, skip_bc2):
    key = (skip_b2, skip_bc2)
    if key not in _PROGRAMS:
        _PROGRAMS[key] = build_program(skip_b2=skip_b2, skip_bc2=skip_bc2)
    return _PROGRAMS[key]


def kernel(**inputs):
    skip_b2 = not np.any(np.asarray(inputs["tb2"]))
    skip_bc2 = not np.any(np.asarray(inputs["cb2"]))
    prog = get_program(skip_b2, skip_bc2)
    in_maps = prep_inputs(**inputs)
    res = run_bass_kernel_spmd(prog, in_maps, list(range(NCORE)))
    return assemble_output(res.results)


if __name__ == "__main__":
    from scipy.special import erf

    rng = np.random.RandomState(0)
    s = 0.02
    inputs = dict(
        x=rng.randn(B, C, N).astype(np.float32),
        g1=np.ones(N, np.float32), be1=np.zeros(N, np.float32),
        g2=np.ones(N, np.float32), be2=np.zeros(N, np.float32),
        tw1=(rng.randn(C, N, N) * s).astype(np.float32),
        tb1=np.zeros((C, N), np.float32),
        tw2=(rng.randn(C, N, N) * s).astype(np.float32),
        tb2=np.zeros((C, N), np.float32),
        cw1=(rng.randn(N, C, C) * s).astype(np.float32),
        cb1=np.zeros((N, C), np.float32),
        cw2=(rng.randn(N, C, C) * s).astype(np.float32),
        cb2=np.zeros((N, C), np.float32),
    )

    def np_ref(x, g1, be1, g2, be2, tw1, tb1, tw2, tb2, cw1, cb1, cw2, cb2):
        def ln(z, g, b):
            mu = z.mean(-1, keepdims=True)
            var = z.var(-1, keepdims=True)
            return (z - mu) / np.sqrt(var + EPS) * g + b
        def gelu(v):
            return v * 0.5 * (1 + erf(v / np.sqrt(2.0)))
        xn = ln(x, g1, be1)
        h = gelu(np.einsum('bcn,cmn->bcm', xn, tw1) + tb1[None])
        tok = np.einsum('bcm,ckm->bck', h, tw2) + tb2[None]
        x = x + tok
        yn = ln(x, g2, be2)
        h2 = gelu(np.einsum('bcn,noc->bon', yn, cw1) + cb1.T[None])
        ch = np.einsum('bon,nko->bkn', h2, cw2) + cb2.T[None]
        return x + ch

    exp = np_ref(**{k: v.astype(np.float64) for k, v in inputs.items()})
    got = kernel(**inputs)
    err = np.abs(got - exp)
    rel = err.max() / np.abs(exp).max()
    print(f"abs err: {err.max():.3e}  rel(absmax): {rel:.3e}")



# revision 7
# speedup vs baseline: 1.7370x; 1.7370x over previous
"""Mixer (token-mix + channel-mix MLP) kernel for 8 TRN2 NeuronCores.

v2 redesign (expert-parallel over group axes, [batch, feature] activations):
  Phase 1 (C-sharded): core m owns channels Cm=[32m,32m+32). LN1 runs in
  [b, n] layout: per-channel stats via big fused DVE reduce ops (chunked by
  8 channels), then xn = (x*rstd + nmr) as one tensor_scalar op per channel
  (per-partition scalar columns). xn is transposed per 128-block by plain
  bf16 matmuls against a shared identity (no fp32 LOW_HIGH passes anywhere).
  fc1 runs weight-stationary (4 matmuls, bias preloaded into PSUM via a
  tiny K=2 matmul so gelu is a single activation op); fc2 runs
  weight-moving (2 matmuls with 256-wide moving operand) producing
  tok[b, n] directly. u = x + tok is one DVE scalar_tensor_tensor with a
  fused sum accumulator; sum(u^2) comes from a scalar-engine Square
  activation with accum_out. yn = (u*rstd2 + nmr2) per channel.
  AllToAll: u and yn ship in bf16, in 4 chunks of 8 channels, each chunk's
  collective issued as soon as its channels are done so the collectives
  overlap the phase-1 tail. Blocks are [dest, {yn,u}, b, c*nl] with 512B+
  contiguous runs so staging DMAs are cheap.
  Phase 2 (N-sharded): core k owns patches Nk. yn arrives ready to use
  (no normalization work): per patch, transpose yn columns to [c, b],
  channel-mix fc1 (weight-stationary, PSUM-preloaded bias) + fc2
  (weight-moving), then y = u2 + ch in one DVE op. y stays [b, nl, c] bf16
  and ships out in 4 chunked DMAs; the host reassembles [B, C, N].
"""
import sys
import numpy as np

sys.path.insert(0, "/opt/trn_rl_repo")

import ml_dtypes
import concourse.bass as bass
import concourse.bacc as bacc
import concourse.tile as tile
from concourse import mybir
from concourse.bass_utils import run_bass_kernel_spmd

F32 = mybir.dt.float32
BF16 = mybir.dt.bfloat16
NCORE = 8
B, C, N = 64, 256, 256
CL = C // NCORE   # 32 local channels (phase 1)
NL = N // NCORE   # 32 local patches (phase 2)
EPS = 1e-5
GELU = mybir.ActivationFunctionType.Gelu
COPY = mybir.ActivationFunctionType.Copy
IDENT = mybir.ActivationFunctionType.Identity
SQRT = mybir.ActivationFunctionType.Sqrt
SQUARE = mybir.ActivationFunctionType.Square
MUL = mybir.AluOpType.mult
ADD = mybir.AluOpType.add
SUB = mybir.AluOpType.subtract

CH = 8                 # channels per stats/collective chunk
NCHUNK = CL // CH      # 4 chunks
BLK = CH * NL          # 256 elems per (dest, tensor) block row


def build_program(gelu_func=GELU, mmdt=BF16, ws=1.0, skip_b2=True,
                  skip_bc2=True):
    nc = bacc.Bacc("TRN2", target_bir_lowering=False, debug=False,
                   enable_asserts=True, num_devices=NCORE)
    wsi = 1.0 / ws

    x_in = nc.dram_tensor("x_sh", [B, CL, N], F32, kind="ExternalInput")
    wt_in = nc.dram_tensor("wt", [CL, 128, 4, N], mmdt, kind="ExternalInput")
    ct_in = nc.dram_tensor("ct", [NL, 128, 4, C], mmdt, kind="ExternalInput")
    b1p_in = nc.dram_tensor("b1p", [2, CL, 128], BF16, kind="ExternalInput")
    bc1p_in = nc.dram_tensor("bc1p", [2, NL, 128], BF16, kind="ExternalInput")
    id64_in = nc.dram_tensor("id64", [64, 64], BF16, kind="ExternalInput")
    mbsel_in = nc.dram_tensor("mbsel", [2, 128], BF16, kind="ExternalInput")
    if not skip_b2:
        b2r_in = nc.dram_tensor("b2r", [CL, N], BF16, kind="ExternalInput")
    if not skip_bc2:
        bc2r_in = nc.dram_tensor("bc2r", [NL, C], BF16, kind="ExternalInput")

    ybuf = nc.dram_tensor("ybuf", [B, NL, C], BF16, kind="ExternalOutput")

    with tile.TileContext(nc) as tc:
        with tc.tile_pool(name="const", bufs=1) as const, \
             tc.tile_pool(name="wpool", bufs=6) as wpool, \
             tc.tile_pool(name="act", bufs=3) as act, \
             tc.tile_pool(name="scr", bufs=2) as scr, \
             tc.tile_pool(name="dram", bufs=1, space="DRAM") as dram, \
             tc.tile_pool(name="ps", bufs=2, space="PSUM") as ps:

            # ---- A2A buffers: 4 chunks of 8 channels; block layout
            # [dest, {yn,u}, b, ch*nl] so every DMA run is >=512B ----
            sends = [dram.tile([NCORE, 2, B, BLK], BF16, name=f"snd{k}")
                     for k in range(NCHUNK)]
            recvs = [dram.tile([NCORE, 2, B, BLK], BF16, name=f"rcv{k}")
                     for k in range(NCHUNK)]

            # ---- constants / persistent tiles ----
            id64 = const.tile([64, 64], BF16)
            nc.sync.dma_start(out=id64[:], in_=id64_in[:])
            mbsel = const.tile([2, 128], BF16)
            nc.sync.dma_start(out=mbsel[:], in_=mbsel_in[:])
            b1p = const.tile([2, CL, 128], BF16)
            nc.sync.dma_start(out=b1p[:], in_=b1p_in[:])
            bc1p = const.tile([2, NL, 128], BF16)
            nc.sync.dma_start(out=bc1p[:], in_=bc1p_in[:])
            if not skip_b2:
                b2r = const.tile([CL, N], BF16)
                nc.sync.dma_start(out=b2r[:], in_=b2r_in[:])
                ones1 = const.tile([1, 64], BF16)
                nc.vector.memset(ones1[:], 1.0)
            if not skip_bc2:
                bc2r = const.tile([NL, C], BF16)
                nc.sync.dma_start(out=bc2r[:], in_=bc2r_in[:])
                ones1c = const.tile([1, 64], BF16)
                nc.vector.memset(ones1c[:], 1.0)
            eps64 = const.tile([64, 1], F32)
            nc.vector.memset(eps64[:], EPS)

            # x in 4 chunk tiles so early channels start after 1/4 load
            x_sb = [const.tile([64, CH, N], F32, name=f"x{k}")
                    for k in range(NCHUNK)]
            for k in range(NCHUNK):
                nc.sync.dma_start(out=x_sb[k][:],
                                  in_=x_in[:, k * CH:(k + 1) * CH, :])

            # per-chunk staging for u / yn in [dest, ci, nl] free layout
            u_sbk = [const.tile([64, NCORE, CH, NL], BF16, name=f"u{k}")
                     for k in range(NCHUNK)]
            yn_sbk = [const.tile([64, NCORE, CH, NL], BF16, name=f"yn{k}")
                      for k in range(NCHUNK)]

            # LN stats tiles [64, CL]
            s1 = const.tile([64, CL], F32)
            s1q = const.tile([64, CL], F32)
            s2 = const.tile([64, CL], F32)
            s2q = const.tile([64, CL], F32)
            mu1 = const.tile([64, CL], F32)
            rstd1 = const.tile([64, CL], F32)
            nmr1 = const.tile([64, CL], F32)
            mu2 = const.tile([64, CL], F32)
            rstd2 = const.tile([64, CL], F32)
            nmr2 = const.tile([64, CL], F32)
            tvar = const.tile([64, CL], F32)
            tsd = const.tile([64, CL], F32)

            xn_sb = [const.tile([64, CH, N], BF16, name=f"xn{k}")
                     for k in range(NCHUNK)]

            def ln_big(k, src, sums, sumsqs):
                """Per-chunk fused stats: sums/sumsqs [64, CH] slices."""
                cs = slice(k * CH, (k + 1) * CH)
                sq = scr.tile([64, CH, N], BF16, tag="sq")
                nc.vector.tensor_mul(out=sq[:], in0=src, in1=src)
                nc.vector.tensor_reduce(out=sums[:, cs], in_=src,
                                        axis=mybir.AxisListType.X, op=ADD)
                nc.vector.tensor_reduce(out=sumsqs[:, cs], in_=sq[:],
                                        axis=mybir.AxisListType.X, op=ADD)

            def ln_batch(k, sums, sumsqs, mu, rstd, nmr):
                cs = slice(k * CH, (k + 1) * CH)
                nc.vector.tensor_scalar_mul(out=mu[:, cs], in0=sums[:, cs],
                                            scalar1=1.0 / N)
                nc.vector.tensor_scalar_mul(out=tvar[:, cs],
                                            in0=sumsqs[:, cs],
                                            scalar1=1.0 / N)
                # tvar = E[z^2] - mu^2  (scalar_tensor_tensor: (mu*-1)*mu+? )
                nc.vector.tensor_mul(out=tsd[:, cs], in0=mu[:, cs],
                                     in1=mu[:, cs])
                nc.vector.tensor_sub(out=tvar[:, cs], in0=tvar[:, cs],
                                     in1=tsd[:, cs])
                nc.scalar.activation(out=tsd[:, cs], in_=tvar[:, cs],
                                     func=SQRT, bias=eps64[:], scale=1.0)
                nc.vector.reciprocal(out=rstd[:, cs], in_=tsd[:, cs])
                nc.vector.scalar_tensor_tensor(
                    out=nmr[:, cs], in0=mu[:, cs], scalar=-1.0,
                    in1=rstd[:, cs], op0=MUL, op1=MUL)

            # LN1 chunk 0 up front; later chunks mid-stream
            ln_big(0, x_sb[0][:], s1, s1q)
            ln_batch(0, s1, s1q, mu1, rstd1, nmr1)

            def stage1_pre(c):
                """xn + transpose + z copy for channel c."""
                k, ci = divmod(c, CH)
                nc.vector.tensor_scalar(
                    out=xn_sb[k][:, ci, :], in0=x_sb[k][:, ci, :],
                    scalar1=rstd1[:, c:c + 1], scalar2=nmr1[:, c:c + 1],
                    op0=MUL, op1=ADD)
                zxp = ps.tile([128, 2, 64], F32, tag="zxp")
                for blk in range(2):
                    nc.tensor.matmul(
                        zxp[:, blk, :],
                        xn_sb[k][:, ci, blk * 128:(blk + 1) * 128],
                        id64[:], start=True, stop=True)
                z_sb = act.tile([128, 2, 64], BF16, tag="z")
                nc.scalar.copy(out=z_sb[:], in_=zxp[:])
                return z_sb

            def mix_block(c, z_sb, w12, bp, b_r, ones_t, skip_bias2,
                          hpre_tag="hpre", tok_tag="tok"):
                """fc1 + gelu + fc2 for one group; returns tok PSUM."""
                hpre = ps.tile([128, 2, 64], F32, tag=hpre_tag)
                # bias preload: out[p, mb, b] = bp[mb, p]
                nc.tensor.matmul(
                    hpre[:].rearrange("p a b -> p (a b)"),
                    bp[:, c, :], mbsel[:],
                    start=True, stop=False, skip_group_check=True)
                for mb in range(2):
                    for nb in range(2):
                        nc.tensor.matmul(
                            hpre[:, mb, :],
                            w12[:, nb, mb * 128:(mb + 1) * 128],
                            z_sb[:, nb, :],
                            start=False, stop=(nb == 1),
                            skip_group_check=True)
                hs = act.tile([128, 2, 64], BF16, tag="h")
                nc.scalar.activation(out=hs[:], in_=hpre[:], func=gelu_func,
                                     scale=wsi)

                tok = ps.tile([64, 256], F32, tag=tok_tag)
                if not skip_bias2:
                    for hb in range(2):
                        nc.tensor.matmul(tok[:, hb * 128:(hb + 1) * 128],
                                         ones_t,
                                         b_r[c:c + 1, hb * 128:(hb + 1) * 128],
                                         start=True, stop=False,
                                         skip_group_check=True)
                for mb in range(2):
                    nc.tensor.matmul(
                        tok[:],
                        hs[:, mb, :],
                        w12[:, 2 + mb, :],
                        start=(skip_bias2 and mb == 0), stop=(mb == 1),
                        skip_group_check=True)
                return tok

            def stage1_post(c, tok):
                """u residual + LN2 partial stats + (chunk tail: yn+send)."""
                k, ci = divmod(c, CH)
                tok3 = tok.rearrange("p (d n) -> p d n", d=NCORE)
                x3 = x_sb[k][:, ci, :].rearrange("p (d n) -> p d n", d=NCORE)
                nc.vector.scalar_tensor_tensor(
                    out=u_sbk[k][:, :, ci, :], in0=tok3, scalar=wsi,
                    in1=x3, op0=MUL, op1=ADD, accum_out=s2[:, c:c + 1])
                sqs = scr.tile([64, NCORE, NL], BF16, tag="sqs")
                nc.scalar.activation(out=sqs[:], in_=u_sbk[k][:, :, ci, :],
                                     func=SQUARE,
                                     accum_out=s2q[:, c:c + 1])

            # ---------------- phase 1 main loop ----------------
            z_cur = stage1_pre(0)
            for c in range(CL):
                k, ci = divmod(c, CH)
                w12 = wpool.tile([128, 4, N], mmdt, tag="w")
                nc.sync.dma_start(out=w12[:], in_=wt_in[c])

                # prefetch next channel's stage-1 (keeps PE/scalar busy)
                z_nxt = stage1_pre(c + 1) if c + 1 < CL else None

                tok = mix_block(c, z_cur, w12, b1p,
                                b2r if not skip_b2 else None,
                                ones1 if not skip_b2 else None, skip_b2)
                stage1_post(c, tok)
                z_cur = z_nxt

                # mid-chunk: emit next chunk's LN1 stats work
                if ci == 3 and k + 1 < NCHUNK:
                    ln_big(k + 1, x_sb[k + 1][:], s1, s1q)
                    ln_batch(k + 1, s1, s1q, mu1, rstd1, nmr1)

                # chunk tail: LN2 batch, yn, ship
                if ci == CH - 1:
                    ln_batch(k, s2, s2q, mu2, rstd2, nmr2)
                    for cj in range(CH):
                        cg = k * CH + cj
                        nc.scalar.activation(
                            out=yn_sbk[k][:, :, cj, :],
                            in_=u_sbk[k][:, :, cj, :], func=IDENT,
                            bias=nmr2[:, cg:cg + 1],
                            scale=rstd2[:, cg:cg + 1])
                    nc.sync.dma_start(
                        out=sends[k][:, 0, :, :].rearrange("d b f -> b d f"),
                        in_=yn_sbk[k][:].rearrange("p d c n -> p d (c n)"))
                    nc.sync.dma_start(
                        out=sends[k][:, 1, :, :].rearrange("d b f -> b d f"),
                        in_=u_sbk[k][:].rearrange("p d c n -> p d (c n)"))
                    nc.gpsimd.collective_compute(
                        "AllToAll", mybir.AluOpType.bypass,
                        replica_groups=[list(range(NCORE))],
                        ins=[sends[k].opt()], outs=[recvs[k].opt()])

            # ---------------- phase 2 staging ----------------
            # yn2/u2: [64, src, chunk, c8, nl]  (c-global = src*32+chunk*8+c8)
            yn2 = const.tile([64, NCORE, NCHUNK, CH, NL], BF16)
            u2 = const.tile([64, NCORE, NCHUNK, CH, NL], BF16)
            for k in range(NCHUNK):
                nc.sync.dma_start(
                    out=yn2[:, :, k, :, :].rearrange("p d c n -> p d (c n)"),
                    in_=recvs[k][:, 0, :, :].rearrange("d b f -> b d f"))
                nc.sync.dma_start(
                    out=u2[:, :, k, :, :].rearrange("p d c n -> p d (c n)"),
                    in_=recvs[k][:, 1, :, :].rearrange("d b f -> b d f"))
            y_sbk = [const.tile([64, CH, C], BF16, name=f"y{k}")
                     for k in range(NCHUNK)]

            def stage2_pre(nl):
                z2p = ps.tile([128, 2, 64], F32, tag="zxp")
                for cb in range(2):
                    nc.tensor.matmul(
                        z2p[:, cb, :],
                        yn2[:, 4 * cb:4 * cb + 4, :, :, nl],
                        id64[:], start=True, stop=True)
                z2 = act.tile([128, 2, 64], BF16, tag="z")
                nc.scalar.copy(out=z2[:], in_=z2p[:])
                return z2

            z2_cur = stage2_pre(0)
            for nl in range(NL):
                c12 = wpool.tile([128, 4, C], mmdt, tag="w")
                nc.sync.dma_start(out=c12[:], in_=ct_in[nl])

                z2_nxt = stage2_pre(nl + 1) if nl + 1 < NL else None

                ch_ps = mix_block(nl, z2_cur, c12, bc1p,
                                  bc2r if not skip_bc2 else None,
                                  ones1c if not skip_bc2 else None, skip_bc2)
                k, ni = divmod(nl, CH)
                nc.vector.scalar_tensor_tensor(
                    out=y_sbk[k][:, ni, :],
                    in0=ch_ps,
                    scalar=wsi,
                    in1=u2[:, :, :, :, nl].rearrange("p a b c -> p (a b c)"),
                    op0=MUL, op1=ADD)
                z2_cur = z2_nxt
                if ni == CH - 1:
                    nc.scalar.dma_start(
                        out=ybuf[:, k * CH:(k + 1) * CH, :],
                        in_=y_sbk[k][:])

    nc.finalize()
    return nc


def prep_inputs(x, g1, be1, g2, be2, tw1, tb1, tw2, tb2, cw1, cb1, cw2, cb2,
                mmdt_np=ml_dtypes.bfloat16, ws=1.0):
    """Host-side sharding + weight folding. Returns in_maps for the 8 cores."""
    f = np.float32
    x = np.asarray(x, f)
    g1, be1, g2, be2 = (np.asarray(a, f) for a in (g1, be1, g2, be2))
    tw1, tb1, tw2, tb2 = (np.asarray(a, f) for a in (tw1, tb1, tw2, tb2))
    cw1, cb1, cw2, cb2 = (np.asarray(a, f) for a in (cw1, cb1, cw2, cb2))

    def wcast(a):
        a = a * ws
        if mmdt_np in (ml_dtypes.float8_e4m3, getattr(ml_dtypes, "float8_e4m3fn", None)):
            a = np.clip(a, -240.0, 240.0)
        return a.astype(mmdt_np)

    # token-mix fc1: fold g1 into weights, be1 into bias; lhsT layout [c, n, m]
    w1t = (tw1 * g1[None, None, :]).transpose(0, 2, 1)            # [C, N, M]
    bias1 = (tb1 + np.einsum('n,cmn->cm', be1, tw1)) * ws         # [C, M]
    w2t = tw2.transpose(0, 2, 1)                                  # [c, m, k]
    t1r = w1t.reshape(C, 2, 128, N)
    t2r = w2t.reshape(C, 2, 128, N)
    wt = np.ascontiguousarray(
        np.stack([t1r[:, 0], t1r[:, 1], t2r[:, 0], t2r[:, 1]],
                 axis=2))
    wt = wcast(wt)                                                # [C,128,4,N]

    # channel-mix fc1: fold g2 (per-patch scalar) into cw1, be2 into bias
    c1t = (cw1 * g2[:, None, None]).transpose(0, 2, 1)            # [N, C, O]
    biasc1 = (cb1 + be2[:, None] * cw1.sum(axis=2)) * ws          # [N, O]
    c2t = cw2.transpose(0, 2, 1)                                  # [n, o, k]
    c1r = c1t.reshape(N, 2, 128, C)
    c2r = c2t.reshape(N, 2, 128, C)
    ct = np.ascontiguousarray(
        np.stack([c1r[:, 0], c1r[:, 1], c2r[:, 0], c2r[:, 1]],
                 axis=2))
    ct = wcast(ct)                                                # [N,128,4,C]

    id64 = np.eye(64, dtype=f).astype(ml_dtypes.bfloat16)
    mbsel = np.zeros((2, 128), f)
    mbsel[0, 0:64] = 1.0
    mbsel[1, 64:128] = 1.0
    mbsel = mbsel.astype(ml_dtypes.bfloat16)

    def fold_pair(bm):   # [G, 256] -> [2, G, 128]
        return np.ascontiguousarray(
            bm.reshape(-1, 2, 128).transpose(1, 0, 2)).astype(
                ml_dtypes.bfloat16)

    in_maps = []
    for m in range(NCORE):
        cs = slice(m * CL, (m + 1) * CL)
        ns = slice(m * NL, (m + 1) * NL)
        d = {
            "x_sh": np.ascontiguousarray(x[:, cs, :]),
            "wt": np.ascontiguousarray(wt[cs]),
            "ct": np.ascontiguousarray(ct[ns]),
            "b1p": fold_pair(bias1[cs]),
            "bc1p": fold_pair(biasc1[ns]),
            "id64": id64,
            "mbsel": mbsel,
        }
        if np.any(tb2):
            d["b2r"] = (tb2[cs] * ws).astype(ml_dtypes.bfloat16)
        if np.any(cb2):
            d["bc2r"] = (cb2.T[ns] * ws).astype(ml_dtypes.bfloat16)
        in_maps.append(d)
    return in_maps


def assemble_output(results):
    """results: per-core dicts with 'ybuf' [B, NL, C] -> y [B, C, N]."""
    y = np.empty((B, C, N), np.float32)
    for k in range(NCORE):
        y[:, :, k * NL:(k + 1) * NL] = (
            results[k]["ybuf"].astype(np.float32).transpose(0, 2, 1))
    return y


_PROGRAMS = {}


def get_program(skip_b2, skip_bc2):
    key = (skip_b2, skip_bc2)
    if key not in _PROGRAMS:
        _PROGRAMS[key] = build_program(skip_b2=skip_b2, skip_bc2=skip_bc2)
    return _PROGRAMS[key]


def kernel(**inputs):
    skip_b2 = not np.any(np.asarray(inputs["tb2"]))
    skip_bc2 = not np.any(np.asarray(inputs["cb2"]))
    prog = get_program(skip_b2, skip_bc2)
    in_maps = prep_inputs(**inputs)
    res = run_bass_kernel_spmd(prog, in_maps, list(range(NCORE)))
    return assemble_output(res.results)


if __name__ == "__main__":
    from scipy.special import erf

    rng = np.random.RandomState(0)
    s = 0.02
    inputs = dict(
        x=rng.randn(B, C, N).astype(np.float32),
        g1=np.ones(N, np.float32), be1=np.zeros(N, np.float32),
        g2=np.ones(N, np.float32), be2=np.zeros(N, np.float32),
        tw1=(rng.randn(C, N, N) * s).astype(np.float32),
        tb1=np.zeros((C, N), np.float32),
        tw2=(rng.randn(C, N, N) * s).astype(np.float32),
        tb2=np.zeros((C, N), np.float32),
        cw1=(rng.randn(N, C, C) * s).astype(np.float32),
        cb1=np.zeros((N, C), np.float32),
        cw2=(rng.randn(N, C, C) * s).astype(np.float32),
        cb2=np.zeros((N, C), np.float32),
    )

    def np_ref(x, g1, be1, g2, be2, tw1, tb1, tw2, tb2, cw1, cb1, cw2, cb2):
        def ln(z, g, b):
            mu = z.mean(-1, keepdims=True)
            var = z.var(-1, keepdims=True)
            return (z - mu) / np.sqrt(var + EPS) * g + b
        def gelu(v):
            return v * 0.5 * (1 + erf(v / np.sqrt(2.0)))
        xn = ln(x, g1, be1)
        h = gelu(np.einsum('bcn,cmn->bcm', xn, tw1) + tb1[None])
        tok = np.einsum('bcm,ckm->bck', h, tw2) + tb2[None]
        x = x + tok
        yn = ln(x, g2, be2)
        h2 = gelu(np.einsum('bcn,noc->bon', yn, cw1) + cb1.T[None])
        ch = np.einsum('bon,nko->bkn', h2, cw2) + cb2.T[None]
        return x + ch

    exp = np_ref(**{k: v.astype(np.float64) for k, v in inputs.items()})
    got = kernel(**inputs)
    err = np.abs(got - exp)
    rel = err.max() / np.abs(exp).max()
    print(f"abs err: {err.max():.3e}  rel(absmax): {rel:.3e}")
